# revision 9
# baseline (speedup 1.0000x reference)
"""Trainium2 Bass kernel for nn_Capsule_16484084482446.

Reference math collapses: with cw = softmax(rw, axis=1),
  outputs[b,j,d] = sum_i sum_n cw[b,i,n] * u[b,j,n,d]
                 = sum_n u[b,j,n,d]           (since sum_i cw[b,i,n] == 1)
so the routing loop is a no-op and the final result is
  out = (sum_n x[b,n,:]) @ W   reshaped to (B, 10, 16).

Kernel strategy (data-parallel over batch, 4 batches per core x 8 cores):
  per core: x_shard (4, 4096, 128) viewed as 128 partitions x (128 rows x 128 d);
  partition p holds rows [128p, 128p+128), so batch b owns partitions [32b, 32b+32).

Profile-driven structure (trace: x-stream runs at ~367 GB/s with zero gaps on a
single sync-ring DMA chain; all remaining time is tail + fixed NEFF overhead):
  1. Sync issues chunked HWDGE DMAs back-to-back; chunk sizes taper at the end
     ([...,8,8,4,4]) so the post-stream fold+matmul tail is short.
  2. VectorE folds each chunk with halving adds; the first add narrows fp32 ->
     bf16 (same DVE rate), later levels run bf16 in-place at 2x DVE rate.
  3. PE accumulates each chunk's bf16 red via a single-pass bf16 matmul against
     a 0/1 batch mask -> psum_s[d, b]  (fp32 LOW_HIGH would cost 2x LDW+MM).
  4. psum_s -> s_bf (bf16 cast copy), then one bf16 matmul s^T @ W_bf -> out.
     W loads on the otherwise-idle Scalar (ACT) HWDGE ring and Scalar itself
     casts it to bf16 — NOT via GpSimd SWDGE: a single SWDGE DMA makes SDMA
     engine 15 straggle ~4.5 us on the x-stream (descriptor-ring AXI port
     contention; measured). GpSimd only memsets the masks (off VectorE).
  bf16 only touches the tiny PE contractions (the 16384-row fold stays fp32 on
  DVE): measured rel err ~1e-3 vs the 2e-2 gate.

No in-kernel semaphore clears: the compiler-emitted NEFF epilogue clears every
kernel semaphore (S[3..255]) after each execution, so re-execution (profiler
loops the NEFF) always starts from zeros.

Raw Bass (no TileContext): Tile's tail drain needs more sync-wait slots than the
TRN2 CTRL encoding allows for this DMA-lane mix, and its end-of-kernel barriers
would dominate a ~35 us kernel.
"""

from contextlib import ExitStack

import numpy as np

import concourse.bass as bass
from concourse import mybir
from concourse.bass_utils import run_bass_kernel_spmd

N_CORES = 8
B, N, DIN = 32, 4096, 128
BSH = B // N_CORES          # 4 batches per core
DOUT = 160                  # 10 capsules * 16 dims
# rows-per-partition split; tapered tail so the last chunks' fold+matmul are
# tiny and pipeline against the stream: a 4-row fold (~0.64 us) matches its
# own ~0.62 us arrival window, so only the final 2-row fold trails the stream
CHUNKS = [8, 16, 16, 16, 16, 16, 16, 4, 4, 4, 4, 4, 2, 2]
assert sum(CHUNKS) == BSH * N // 128

F32 = mybir.dt.float32
BF16 = mybir.dt.bfloat16

_cache = {}


def _build_nc(chunks=None, wait_out=False):
    """wait_out: wait for the output DMA's completion sem before the end-of-
    block barrier. The NEFF epilogue (engine barriers + 253 sem clears, ~7 us)
    runs after our last instruction either way, giving the 2.5 KB output write
    ample time to land before the runtime reads it back."""
    chunks = CHUNKS if chunks is None else chunks
    assert sum(chunks) == BSH * N // 128
    nchunk = len(chunks)
    nc = bass.Bass()
    x = nc.dram_tensor("x", [BSH, N, DIN], F32, kind="ExternalInput")
    w = nc.dram_tensor("W", [DIN, DOUT], F32, kind="ExternalInput")
    out = nc.dram_tensor("out", [BSH, DOUT], F32, kind="ExternalOutput")

    # (128, 128, 128): partition p, row-in-partition n, feature d
    x3 = x[:].flatten_outer_dims().rearrange("(p n) d -> p n d", p=128)
    starts = np.cumsum([0] + chunks).tolist()

    with ExitStack() as ctx:
        ec = ctx.enter_context
        xc = [ec(nc.sbuf_tensor(f"xc{c}", [128, chunks[c] * DIN], F32))
              for c in range(nchunk)]
        # bf16 fold buffers: first halving add writes here, then in-place
        red = [ec(nc.sbuf_tensor(f"red{c}", [128, max(chunks[c] // 2, 1) * DIN],
                                 BF16))
               for c in range(nchunk)]
        w_sb = ec(nc.sbuf_tensor("w_sb", [DIN, DOUT], F32))
        w_bf = ec(nc.sbuf_tensor("w_bf", [DIN, DOUT], BF16))
        mask_bf = ec(nc.sbuf_tensor("mask_bf", [128, BSH], BF16))
        s_bf = ec(nc.sbuf_tensor("s_bf", [DIN, BSH], BF16))
        out_sb = ec(nc.sbuf_tensor("out_sb", [BSH, DOUT], F32))
        psum_s = ec(nc.psum_tensor("psum_s", [DIN, BSH], F32))
        psum_o = ec(nc.psum_tensor("psum_o", [BSH, DOUT], F32))

        dma_w = ec(nc.semaphore("dma_w"))
        w_ready = ec(nc.semaphore("w_ready"))
        g_mask = ec(nc.semaphore("g_mask"))
        dma_c = [ec(nc.semaphore(f"dma_c{c}")) for c in range(nchunk)]
        v_red = ec(nc.semaphore("v_red"))    # +1 per finished red[c]
        pe_sem = ec(nc.semaphore("pe_sem"))
        v_sem = ec(nc.semaphore("v_sem"))    # s_bf ready
        v_out = ec(nc.semaphore("v_out"))
        dma_out = ec(nc.semaphore("dma_out"))
        block = ec(nc.Block())

        @block.sync
        def _(sync):
            for c in range(nchunk):
                sync.dma_start(
                    xc[c][:], x3[:, starts[c] : starts[c + 1], :]
                ).then_inc(dma_c[c], 16)
            sync.wait_ge(v_out, 1)
            sync.dma_start(out[:], out_sb[:]).then_inc(dma_out, 16)
            if wait_out:
                sync.wait_ge(dma_out, 16)

        @block.scalar
        def _(scalar):
            # W only feeds the final tiny matmul; the ACT HWDGE ring keeps it
            # off the sync ring, and Scalar does the bf16 cast itself.
            scalar.dma_start(w_sb[:], w[:]).then_inc(dma_w, 16)
            scalar.wait_ge(dma_w, 16)
            scalar.copy(w_bf[:], w_sb[:]).then_inc(w_ready, 1)

        @block.gpsimd
        def _(gpsimd):
            # 0/1 batch mask, one 32-partition quadrant at a time (nonzero
            # partition bases only allow 32-partition windows)
            op = None
            for q in range(4):
                for b in range(BSH):
                    op = gpsimd.memset(
                        mask_bf[32 * q : 32 * (q + 1), b : b + 1],
                        1.0 if q == b else 0.0,
                    )
            op.then_inc(g_mask, 1)

        @block.vector
        def _(vector):
            for c in range(nchunk):
                vector.wait_ge(dma_c[c], 16)
                rows = chunks[c]
                if rows == 1:
                    op = vector.tensor_copy(red[c][:, :DIN], xc[c][:, :DIN])
                else:
                    half = rows // 2 * DIN
                    # fp32 -> bf16 narrowing add, then 2x-rate bf16 halvings
                    op = vector.tensor_add(
                        red[c][:, :half], xc[c][:, :half],
                        xc[c][:, half : 2 * half],
                    )
                    s = half
                    while s > DIN:
                        s //= 2
                        op = vector.tensor_add(
                            red[c][:, :s], red[c][:, :s], red[c][:, s : 2 * s]
                        )
                op.then_inc(v_red, 1)
            vector.wait_ge(pe_sem, 1)
            vector.tensor_copy(s_bf[:], psum_s[:]).then_inc(v_sem, 1)
            vector.wait_ge(pe_sem, 2)
            vector.tensor_copy(out_sb[:], psum_o[:]).then_inc(v_out, 1)

        @block.tensor
        def _(tensor):
            tensor.wait_ge(g_mask, 1)
            # s[d, b] += sum_p red_c[p, d] * mask[p, b], accumulated over chunks
            for c in range(nchunk):
                tensor.wait_ge(v_red, c + 1)
                mm = tensor.matmul(
                    psum_s[:],
                    red[c][:, :DIN],
                    mask_bf[:],
                    start=(c == 0),
                    stop=(c == nchunk - 1),
                )
            mm.then_inc(pe_sem, 1)
            tensor.wait_ge(w_ready, 1)
            tensor.wait_ge(v_sem, 1)
            # out[b, jd] = sum_d s[d, b] * W[d, jd]
            tensor.matmul(
                psum_o[:], s_bf[:], w_bf[:], start=True, stop=True
            ).then_inc(pe_sem, 1)

    return nc


def _get_nc():
    if "nc" not in _cache:
        _cache["nc"] = _build_nc()
    return _cache["nc"]


def _in_maps(x, W):
    x = np.ascontiguousarray(x, dtype=np.float32)
    W = np.ascontiguousarray(W, dtype=np.float32)
    return [{"x": x[i * BSH : (i + 1) * BSH], "W": W} for i in range(N_CORES)]


def kernel(x, W, **profile_kwargs):
    nc = _get_nc()
    res = run_bass_kernel_spmd(nc, _in_maps(x, W), list(range(N_CORES)), **profile_kwargs)
    out = np.concatenate([r["out"] for r in res.results], axis=0)
    ret = out.reshape(B, 10, 16).astype(np.float32)
    if profile_kwargs:
        ret = (ret, res)
    return ret


# revision 12
# speedup vs baseline: 1.0290x; 1.0290x over previous
"""Trainium2 Bass kernel for nn_Capsule_16484084482446.

Reference math collapses: with cw = softmax(rw, axis=1),
  outputs[b,j,d] = sum_i sum_n cw[b,i,n] * u[b,j,n,d]
                 = sum_n u[b,j,n,d]           (since sum_i cw[b,i,n] == 1)
so the routing loop is a no-op and the final result is
  out = (sum_n x[b,n,:]) @ W   reshaped to (B, 10, 16).

Kernel strategy (data-parallel over batch, 4 batches per core x 8 cores):
  per core: x_shard (4, 4096, 128) viewed as 128 partitions x (128 rows x 128 d);
  partition p holds rows [128p, 128p+128), so batch b owns partitions [32b, 32b+32).

Profile-driven structure (trace: x-stream runs at ~367 GB/s with zero gaps on a
single sync-ring DMA chain; all remaining time is tail + fixed NEFF overhead):
  1. Sync issues chunked HWDGE DMAs back-to-back; chunk sizes taper at the end
     ([...,8,8,4,4]) so the post-stream fold+matmul tail is short.
  2. VectorE folds each chunk with halving adds; the first add narrows fp32 ->
     bf16 (same DVE rate), later levels run bf16 in-place at 2x DVE rate.
  3. PE accumulates each chunk's bf16 red via a single-pass bf16 matmul against
     a 0/1 batch mask -> psum_s[d, b]  (fp32 LOW_HIGH would cost 2x LDW+MM).
  4. psum_s -> s_bf (bf16 cast copy), then one bf16 matmul s^T @ W_bf -> out.
     W loads on the otherwise-idle Scalar (ACT) HWDGE ring and Scalar itself
     casts it to bf16 — NOT via GpSimd SWDGE: a single SWDGE DMA makes SDMA
     engine 15 straggle ~4.5 us on the x-stream (descriptor-ring AXI port
     contention; measured). GpSimd only memsets the masks (off VectorE).
  bf16 only touches the tiny PE contractions (the 16384-row fold stays fp32 on
  DVE): measured rel err ~1e-3 vs the 2e-2 gate.

No in-kernel semaphore clears: the compiler-emitted NEFF epilogue clears every
kernel semaphore (S[3..255]) after each execution, so re-execution (profiler
loops the NEFF) always starts from zeros.

Raw Bass (no TileContext): Tile's tail drain needs more sync-wait slots than the
TRN2 CTRL encoding allows for this DMA-lane mix, and its end-of-kernel barriers
would dominate a ~35 us kernel.
"""

from contextlib import ExitStack

import numpy as np

import concourse.bass as bass
from concourse import mybir
from concourse.bass_utils import run_bass_kernel_spmd

N_CORES = 8
B, N, DIN = 32, 4096, 128
BSH = B // N_CORES          # 4 batches per core
DOUT = 160                  # 10 capsules * 16 dims
# rows-per-partition split; tapered tail so the last chunks' fold+matmul are
# tiny and pipeline against the stream: a 4-row fold (~0.64 us) matches its
# own ~0.62 us arrival window, so only the final 2-row fold trails the stream
CHUNKS = [8, 16, 16, 16, 16, 16, 16, 4, 4, 4, 4, 4, 2, 2]
assert sum(CHUNKS) == BSH * N // 128

F32 = mybir.dt.float32
BF16 = mybir.dt.bfloat16

_cache = {}


def _red_rows(rows):
    """How many rows a chunk's DVE fold leaves for PE to contract."""
    return 1 if rows <= 2 else 2


def _build_nc(chunks=None, wait_out=False):
    """wait_out: wait for the output DMA's completion sem before the end-of-
    block barrier. The NEFF epilogue (engine barriers + 253 sem clears, ~7 us)
    runs after our last instruction either way, giving the 2.5 KB output write
    ample time to land before the runtime reads it back."""
    chunks = CHUNKS if chunks is None else chunks
    assert sum(chunks) == BSH * N // 128
    nchunk = len(chunks)
    nc = bass.Bass()
    x = nc.dram_tensor("x", [BSH, N, DIN], F32, kind="ExternalInput")
    w = nc.dram_tensor("W", [DIN, DOUT], F32, kind="ExternalInput")
    out = nc.dram_tensor("out", [BSH, DOUT], F32, kind="ExternalOutput")

    # (128, 128, 128): partition p, row-in-partition n, feature d
    x3 = x[:].flatten_outer_dims().rearrange("(p n) d -> p n d", p=128)
    starts = np.cumsum([0] + chunks).tolist()

    with ExitStack() as ctx:
        ec = ctx.enter_context
        xc = [ec(nc.sbuf_tensor(f"xc{c}", [128, chunks[c] * DIN], F32))
              for c in range(nchunk)]
        # bf16 fold buffers: first halving add writes here, then in-place
        red = [ec(nc.sbuf_tensor(f"red{c}", [128, max(chunks[c] // 2, 1) * DIN],
                                 BF16))
               for c in range(nchunk)]
        w_sb = ec(nc.sbuf_tensor("w_sb", [DIN, DOUT], F32))
        w_bf = ec(nc.sbuf_tensor("w_bf", [DIN, DOUT], BF16))
        mask_bf = ec(nc.sbuf_tensor("mask_bf", [128, BSH], BF16))
        s_bf = ec(nc.sbuf_tensor("s_bf", [DIN, BSH], BF16))
        out_sb = ec(nc.sbuf_tensor("out_sb", [BSH, DOUT], F32))
        psum_s = ec(nc.psum_tensor("psum_s", [DIN, BSH], F32))
        psum_o = ec(nc.psum_tensor("psum_o", [BSH, DOUT], F32))

        dma_w = ec(nc.semaphore("dma_w"))
        w_ready = ec(nc.semaphore("w_ready"))
        g_mask = ec(nc.semaphore("g_mask"))
        dma_c = [ec(nc.semaphore(f"dma_c{c}")) for c in range(nchunk)]
        v_red = ec(nc.semaphore("v_red"))    # +1 per finished red[c]
        pe_sem = ec(nc.semaphore("pe_sem"))
        v_sem = ec(nc.semaphore("v_sem"))    # s_bf ready
        v_out = ec(nc.semaphore("v_out"))
        dma_out = ec(nc.semaphore("dma_out"))
        block = ec(nc.Block())

        @block.sync
        def _(sync):
            for c in range(nchunk):
                sync.dma_start(
                    xc[c][:], x3[:, starts[c] : starts[c + 1], :]
                ).then_inc(dma_c[c], 16)
            sync.wait_ge(v_out, 1)
            sync.dma_start(out[:], out_sb[:]).then_inc(dma_out, 16)
            if wait_out:
                sync.wait_ge(dma_out, 16)

        @block.scalar
        def _(scalar):
            # W only feeds the final tiny matmul; the ACT HWDGE ring keeps it
            # off the sync ring, and Scalar does the bf16 cast itself.
            scalar.dma_start(w_sb[:], w[:]).then_inc(dma_w, 16)
            scalar.wait_ge(dma_w, 16)
            scalar.copy(w_bf[:], w_sb[:]).then_inc(w_ready, 1)

        @block.gpsimd
        def _(gpsimd):
            # 0/1 batch mask, one 32-partition quadrant at a time (nonzero
            # partition bases only allow 32-partition windows)
            op = None
            for q in range(4):
                for b in range(BSH):
                    op = gpsimd.memset(
                        mask_bf[32 * q : 32 * (q + 1), b : b + 1],
                        1.0 if q == b else 0.0,
                    )
            op.then_inc(g_mask, 1)

        @block.vector
        def _(vector):
            for c in range(nchunk):
                vector.wait_ge(dma_c[c], 16)
                rows = chunks[c]
                half = rows // 2 * DIN
                # fp32 -> bf16 narrowing add, then 2x-rate bf16 halvings.
                # Stop at TWO rows (PE eats the last level as a second cheap
                # bf16 matmul) so tail folds stay under the chunk arrival rate.
                op = vector.tensor_add(
                    red[c][:, :half], xc[c][:, :half],
                    xc[c][:, half : 2 * half],
                )
                s = half
                while s > _red_rows(rows) * DIN:
                    s //= 2
                    op = vector.tensor_add(
                        red[c][:, :s], red[c][:, :s], red[c][:, s : 2 * s]
                    )
                op.then_inc(v_red, 1)
            vector.wait_ge(pe_sem, 1)
            vector.tensor_copy(s_bf[:], psum_s[:]).then_inc(v_sem, 1)
            vector.wait_ge(pe_sem, 2)
            vector.tensor_copy(out_sb[:], psum_o[:]).then_inc(v_out, 1)

        @block.tensor
        def _(tensor):
            tensor.wait_ge(g_mask, 1)
            # s[d, b] += sum_p red_c[p, d] * mask[p, b], accumulated over chunks
            # (one matmul per remaining red row)
            first = True
            for c in range(nchunk):
                tensor.wait_ge(v_red, c + 1)
                for r in range(_red_rows(chunks[c])):
                    mm = tensor.matmul(
                        psum_s[:],
                        red[c][:, r * DIN : (r + 1) * DIN],
                        mask_bf[:],
                        start=first,
                        stop=(c == nchunk - 1
                              and r == _red_rows(chunks[c]) - 1),
                    )
                    first = False
            mm.then_inc(pe_sem, 1)
            tensor.wait_ge(w_ready, 1)
            tensor.wait_ge(v_sem, 1)
            # out[b, jd] = sum_d s[d, b] * W[d, jd]
            tensor.matmul(
                psum_o[:], s_bf[:], w_bf[:], start=True, stop=True
            ).then_inc(pe_sem, 1)

    return nc


def _get_nc():
    if "nc" not in _cache:
        _cache["nc"] = _build_nc()
    return _cache["nc"]


def _in_maps(x, W):
    x = np.ascontiguousarray(x, dtype=np.float32)
    W = np.ascontiguousarray(W, dtype=np.float32)
    return [{"x": x[i * BSH : (i + 1) * BSH], "W": W} for i in range(N_CORES)]


def kernel(x, W, **profile_kwargs):
    nc = _get_nc()
    res = run_bass_kernel_spmd(nc, _in_maps(x, W), list(range(N_CORES)), **profile_kwargs)
    out = np.concatenate([r["out"] for r in res.results], axis=0)
    ret = out.reshape(B, 10, 16).astype(np.float32)
    if profile_kwargs:
        ret = (ret, res)
    return ret


# revision 15
# speedup vs baseline: 1.0348x; 1.0056x over previous
"""Trainium2 Bass kernel for nn_Capsule_16484084482446.

Reference math collapses: with cw = softmax(rw, axis=1),
  outputs[b,j,d] = sum_i sum_n cw[b,i,n] * u[b,j,n,d]
                 = sum_n u[b,j,n,d]           (since sum_i cw[b,i,n] == 1)
so the routing loop is a no-op and the final result is
  out = (sum_n x[b,n,:]) @ W   reshaped to (B, 10, 16).

Kernel strategy (data-parallel over batch, 4 batches per core x 8 cores):
  per core: x_shard (4, 4096, 128) viewed as 128 partitions x (128 rows x 128 d);
  partition p holds rows [128p, 128p+128), so batch b owns partitions [32b, 32b+32).

Profile-driven structure (trace: x-stream runs at ~367 GB/s with zero gaps on a
single sync-ring DMA chain; all remaining time is tail + fixed NEFF overhead):
  1. Sync issues chunked HWDGE DMAs back-to-back; chunk sizes taper at the end
     ([...,8,8,4,4]) so the post-stream fold+matmul tail is short.
  2. VectorE folds each chunk with halving adds; the first add narrows fp32 ->
     bf16 (same DVE rate), later levels run bf16 in-place at 2x DVE rate.
  3. PE accumulates each chunk's bf16 red via a single-pass bf16 matmul against
     a 0/1 batch mask -> psum_s[d, b]  (fp32 LOW_HIGH would cost 2x LDW+MM).
  4. psum_s -> s_bf (bf16 cast copy), then one bf16 matmul s^T @ W_bf -> out.
     W loads on the otherwise-idle Scalar (ACT) HWDGE ring and Scalar itself
     casts it to bf16 — NOT via GpSimd SWDGE: a single SWDGE DMA makes SDMA
     engine 15 straggle ~4.5 us on the x-stream (descriptor-ring AXI port
     contention; measured). GpSimd only memsets the masks (off VectorE).
  bf16 only touches the tiny PE contractions (the 16384-row fold stays fp32 on
  DVE): measured rel err ~1e-3 vs the 2e-2 gate.

No in-kernel semaphore clears: the compiler-emitted NEFF epilogue clears every
kernel semaphore (S[3..255]) after each execution, so re-execution (profiler
loops the NEFF) always starts from zeros.

Raw Bass (no TileContext): Tile's tail drain needs more sync-wait slots than the
TRN2 CTRL encoding allows for this DMA-lane mix, and its end-of-kernel barriers
would dominate a ~35 us kernel.
"""

from contextlib import ExitStack

import numpy as np

import concourse.bass as bass
from concourse import mybir
from concourse.bass_utils import run_bass_kernel_spmd

N_CORES = 8
B, N, DIN = 32, 4096, 128
BSH = B // N_CORES          # 4 batches per core
DOUT = 160                  # 10 capsules * 16 dims
# rows-per-partition split; tapered tail so the last chunks' fold+matmul are
# tiny and pipeline against the stream: a 4-row fold (~0.64 us) matches its
# own ~0.62 us arrival window, so only the final 2-row fold trails the stream
CHUNKS = [8, 16, 16, 16, 16, 16, 16, 4, 4, 4, 4, 4, 4]
assert sum(CHUNKS) == BSH * N // 128

F32 = mybir.dt.float32
BF16 = mybir.dt.bfloat16

_cache = {}


def _red_rows(rows):
    """How many rows a chunk's DVE fold leaves for PE to contract."""
    return 1 if rows <= 2 else 2


class _NoBarrierBlock(bass.BassBlock):
    """BassBlock minus the exit all_engine_barrier: the compiler's NEFF
    epilogue starts with its own all-engine barrier, so bass's is a redundant
    ~0.5 us serial gather/release on the measured critical path."""

    def __exit__(self, exc_type, exc_val, exc_tb):
        if exc_type is not None:
            return
        for engine, last_body in self.last_body.items():
            with self.bass.body(
                last_body, parent=self.bass.cur_bb, allow_existing_parent=True
            ):
                engine.br(self.end_bb)
        self.bass.switch_bb(self.end_bb)


def _build_nc(chunks=None, wait_out=False):
    """wait_out: wait for the output DMA's completion sem before the end-of-
    block barrier. The NEFF epilogue (engine barriers + 253 sem clears, ~7 us)
    runs after our last instruction either way, giving the 2.5 KB output write
    ample time to land before the runtime reads it back."""
    chunks = CHUNKS if chunks is None else chunks
    assert sum(chunks) == BSH * N // 128
    nchunk = len(chunks)
    nc = bass.Bass()
    x = nc.dram_tensor("x", [BSH, N, DIN], F32, kind="ExternalInput")
    w = nc.dram_tensor("W", [DIN, DOUT], F32, kind="ExternalInput")
    out = nc.dram_tensor("out", [BSH, DOUT], F32, kind="ExternalOutput")

    # (128, 128, 128): partition p, row-in-partition n, feature d
    x3 = x[:].flatten_outer_dims().rearrange("(p n) d -> p n d", p=128)
    starts = np.cumsum([0] + chunks).tolist()

    with ExitStack() as ctx:
        ec = ctx.enter_context
        xc = [ec(nc.sbuf_tensor(f"xc{c}", [128, chunks[c] * DIN], F32))
              for c in range(nchunk)]
        # bf16 fold buffers: first halving add writes here, then in-place
        red = [ec(nc.sbuf_tensor(f"red{c}", [128, max(chunks[c] // 2, 1) * DIN],
                                 BF16))
               for c in range(nchunk)]
        w_sb = ec(nc.sbuf_tensor("w_sb", [DIN, DOUT], F32))
        w_bf = ec(nc.sbuf_tensor("w_bf", [DIN, DOUT], BF16))
        mask_bf = ec(nc.sbuf_tensor("mask_bf", [128, BSH], BF16))
        s_bf = ec(nc.sbuf_tensor("s_bf", [DIN, BSH], BF16))
        out_sb = ec(nc.sbuf_tensor("out_sb", [BSH, DOUT], F32))
        psum_s = ec(nc.psum_tensor("psum_s", [DIN, BSH], F32))
        psum_o = ec(nc.psum_tensor("psum_o", [BSH, DOUT], F32))

        dma_w = ec(nc.semaphore("dma_w"))
        w_ready = ec(nc.semaphore("w_ready"))
        g_mask = ec(nc.semaphore("g_mask"))
        dma_c = [ec(nc.semaphore(f"dma_c{c}")) for c in range(nchunk)]
        v_red = ec(nc.semaphore("v_red"))    # +1 per finished red[c]
        pe_sem = ec(nc.semaphore("pe_sem"))
        v_sem = ec(nc.semaphore("v_sem"))    # s_bf ready
        v_out = ec(nc.semaphore("v_out"))
        dma_out = ec(nc.semaphore("dma_out"))
        block = ec(_NoBarrierBlock(nc, f"block_{nc.next_id()}"))

        @block.sync
        def _(sync):
            for c in range(nchunk):
                sync.dma_start(
                    xc[c][:], x3[:, starts[c] : starts[c + 1], :]
                ).then_inc(dma_c[c], 16)
            sync.wait_ge(v_out, 1)
            sync.dma_start(out[:], out_sb[:]).then_inc(dma_out, 16)
            if wait_out:
                sync.wait_ge(dma_out, 16)

        @block.scalar
        def _(scalar):
            # W only feeds the final tiny matmul; the ACT HWDGE ring keeps it
            # off the sync ring, and Scalar does the bf16 cast itself.
            scalar.dma_start(w_sb[:], w[:]).then_inc(dma_w, 16)
            scalar.wait_ge(dma_w, 16)
            scalar.copy(w_bf[:], w_sb[:]).then_inc(w_ready, 1)

        @block.gpsimd
        def _(gpsimd):
            # 0/1 batch mask, one 32-partition quadrant at a time (nonzero
            # partition bases only allow 32-partition windows)
            op = None
            for q in range(4):
                for b in range(BSH):
                    op = gpsimd.memset(
                        mask_bf[32 * q : 32 * (q + 1), b : b + 1],
                        1.0 if q == b else 0.0,
                    )
            op.then_inc(g_mask, 1)

        @block.vector
        def _(vector):
            for c in range(nchunk):
                vector.wait_ge(dma_c[c], 16)
                rows = chunks[c]
                half = rows // 2 * DIN
                # fp32 -> bf16 narrowing add, then 2x-rate bf16 halvings.
                # Stop at TWO rows (PE eats the last level as a second cheap
                # bf16 matmul) so tail folds stay under the chunk arrival rate.
                op = vector.tensor_add(
                    red[c][:, :half], xc[c][:, :half],
                    xc[c][:, half : 2 * half],
                )
                s = half
                while s > _red_rows(rows) * DIN:
                    s //= 2
                    op = vector.tensor_add(
                        red[c][:, :s], red[c][:, :s], red[c][:, s : 2 * s]
                    )
                op.then_inc(v_red, 1)
            vector.wait_ge(pe_sem, 1)
            vector.tensor_copy(s_bf[:], psum_s[:]).then_inc(v_sem, 1)
            vector.wait_ge(pe_sem, 2)
            vector.tensor_copy(out_sb[:], psum_o[:]).then_inc(v_out, 1)

        @block.tensor
        def _(tensor):
            tensor.wait_ge(g_mask, 1)
            # s[d, b] += sum_p red_c[p, d] * mask[p, b], accumulated over chunks
            # (one matmul per remaining red row)
            first = True
            for c in range(nchunk):
                tensor.wait_ge(v_red, c + 1)
                for r in range(_red_rows(chunks[c])):
                    mm = tensor.matmul(
                        psum_s[:],
                        red[c][:, r * DIN : (r + 1) * DIN],
                        mask_bf[:],
                        start=first,
                        stop=(c == nchunk - 1
                              and r == _red_rows(chunks[c]) - 1),
                    )
                    first = False
            mm.then_inc(pe_sem, 1)
            tensor.wait_ge(w_ready, 1)
            tensor.wait_ge(v_sem, 1)
            # out[b, jd] = sum_d s[d, b] * W[d, jd]
            tensor.matmul(
                psum_o[:], s_bf[:], w_bf[:], start=True, stop=True
            ).then_inc(pe_sem, 1)

    return nc


def _get_nc():
    if "nc" not in _cache:
        _cache["nc"] = _build_nc()
    return _cache["nc"]


def _in_maps(x, W):
    x = np.ascontiguousarray(x, dtype=np.float32)
    W = np.ascontiguousarray(W, dtype=np.float32)
    return [{"x": x[i * BSH : (i + 1) * BSH], "W": W} for i in range(N_CORES)]


def kernel(x, W, **profile_kwargs):
    nc = _get_nc()
    res = run_bass_kernel_spmd(nc, _in_maps(x, W), list(range(N_CORES)), **profile_kwargs)
    out = np.concatenate([r["out"] for r in res.results], axis=0)
    ret = out.reshape(B, 10, 16).astype(np.float32)
    if profile_kwargs:
        ret = (ret, res)
    return ret


# revision 17
# speedup vs baseline: 1.0428x; 1.0077x over previous
"""Trainium2 Bass kernel for nn_Capsule_16484084482446.

Reference math collapses: with cw = softmax(rw, axis=1),
  outputs[b,j,d] = sum_i sum_n cw[b,i,n] * u[b,j,n,d]
                 = sum_n u[b,j,n,d]           (since sum_i cw[b,i,n] == 1)
so the routing loop is a no-op and the final result is
  out = (sum_n x[b,n,:]) @ W   reshaped to (B, 10, 16).

Kernel strategy (data-parallel over batch, 4 batches per core x 8 cores):
  per core: x_shard (4, 4096, 128) viewed as 128 partitions x (128 rows x 128 d);
  partition p holds rows [128p, 128p+128), so batch b owns partitions [32b, 32b+32).

Profile-driven structure (trace: the x-stream saturates the 16 SDMA engines at
~26 GB/s each = 400-430 GB/s aggregate with zero gaps on a single sync-ring DMA
chain; all remaining time is startup + tail + fixed NEFF overhead):
  1. Sync issues chunked HWDGE DMAs back-to-back; chunk sizes taper to 4 rows
     at the end so tail folds pipeline against the last arrivals.
  2. VectorE folds each chunk with halving adds down to TWO rows; the first
     add narrows fp32 -> bf16, later levels run bf16 at ~1.5x DVE rate. PE
     eats the final fold level as a second (cheap) matmul per chunk.
  3. PE accumulates each red row via a single-pass bf16 matmul against a 0/1
     batch mask -> psum_s[d, b]  (fp32 LOW_HIGH would cost 2x LDW+MM, ~5x
     time); LDW+MM measure ~270 ns per row.
  4. psum_s -> s_bf (bf16 cast copy), then one bf16 matmul s^T @ W_bf -> out.
     W loads on the otherwise-idle Scalar (ACT) HWDGE ring and Scalar itself
     casts it to bf16 — NOT via GpSimd SWDGE: a single SWDGE DMA makes SDMA
     engine 15 straggle ~4.5 us on the x-stream (descriptor-ring AXI port
     contention; measured). GpSimd only memsets the masks (off VectorE).
  bf16 only touches the tiny PE contractions (the 16384-row fold's wide levels
  stay fp32 on DVE): measured rel err ~4e-3 vs the 2e-2 gate.

End-game: sync issues the output DMA and exits WITHOUT waiting for its
completion sem, and the block exits WITHOUT bass's all-engine barrier
(_NoBarrierBlock) — the compiler-emitted NEFF epilogue (own all-engine
barrier, then ~6 us of per-semaphore clears of S[3..255], on the measured
critical path every execution) both covers the 2.5 KB write's landing time
and makes in-kernel semaphore clears and the extra barrier redundant.

Known variance: SDMA engine 15 intermittently runs ~20% slower (device state,
not kernel-controlled); afflicted runs measure ~+3.5 us.

Raw Bass (no TileContext): Tile's tail drain needs more sync-wait slots than
the TRN2 CTRL encoding allows for this DMA-lane mix, and its end-of-kernel
barriers would dominate a ~35 us kernel.
"""

from contextlib import ExitStack

import numpy as np

import concourse.bass as bass
from concourse import mybir
from concourse.bass_utils import run_bass_kernel_spmd

N_CORES = 8
B, N, DIN = 32, 4096, 128
BSH = B // N_CORES          # 4 batches per core
DOUT = 160                  # 10 capsules * 16 dims
# rows-per-partition split; tapered tail so the last chunks' fold+matmul are
# tiny and pipeline against the stream: a 4-row fold-to-2 (~0.43 us) fits its
# own ~0.6 us arrival window, so only the final chunk's fold trails the stream
CHUNKS = [8, 16, 16, 16, 16, 16, 16, 4, 4, 4, 4, 4, 4]
assert sum(CHUNKS) == BSH * N // 128

F32 = mybir.dt.float32
BF16 = mybir.dt.bfloat16

_cache = {}


def _red_rows(rows):
    """How many rows a chunk's DVE fold leaves for PE to contract."""
    return 1 if rows <= 2 else 2


class _NoBarrierBlock(bass.BassBlock):
    """BassBlock minus the exit all_engine_barrier: the compiler's NEFF
    epilogue starts with its own all-engine barrier, so bass's is a redundant
    ~0.5 us serial gather/release on the measured critical path."""

    def __exit__(self, exc_type, exc_val, exc_tb):
        if exc_type is not None:
            return
        for engine, last_body in self.last_body.items():
            with self.bass.body(
                last_body, parent=self.bass.cur_bb, allow_existing_parent=True
            ):
                engine.br(self.end_bb)
        self.bass.switch_bb(self.end_bb)


def _build_nc(chunks=None, wait_out=False):
    """wait_out: wait for the output DMA's completion sem before the end-of-
    block barrier. The NEFF epilogue (engine barriers + 253 sem clears, ~7 us)
    runs after our last instruction either way, giving the 2.5 KB output write
    ample time to land before the runtime reads it back."""
    chunks = CHUNKS if chunks is None else chunks
    assert sum(chunks) == BSH * N // 128
    nchunk = len(chunks)
    nc = bass.Bass()
    x = nc.dram_tensor("x", [BSH, N, DIN], F32, kind="ExternalInput")
    w = nc.dram_tensor("W", [DIN, DOUT], F32, kind="ExternalInput")
    out = nc.dram_tensor("out", [BSH, DOUT], F32, kind="ExternalOutput")

    # (128, 128, 128): partition p, row-in-partition n, feature d
    x3 = x[:].flatten_outer_dims().rearrange("(p n) d -> p n d", p=128)
    starts = np.cumsum([0] + chunks).tolist()

    with ExitStack() as ctx:
        ec = ctx.enter_context
        xc = [ec(nc.sbuf_tensor(f"xc{c}", [128, chunks[c] * DIN], F32))
              for c in range(nchunk)]
        # bf16 fold buffers: first halving add writes here, then in-place
        red = [ec(nc.sbuf_tensor(f"red{c}", [128, max(chunks[c] // 2, 1) * DIN],
                                 BF16))
               for c in range(nchunk)]
        w_sb = ec(nc.sbuf_tensor("w_sb", [DIN, DOUT], F32))
        w_bf = ec(nc.sbuf_tensor("w_bf", [DIN, DOUT], BF16))
        mask_bf = ec(nc.sbuf_tensor("mask_bf", [128, BSH], BF16))
        s_bf = ec(nc.sbuf_tensor("s_bf", [DIN, BSH], BF16))
        out_sb = ec(nc.sbuf_tensor("out_sb", [BSH, DOUT], F32))
        psum_s = ec(nc.psum_tensor("psum_s", [DIN, BSH], F32))
        psum_o = ec(nc.psum_tensor("psum_o", [BSH, DOUT], F32))

        dma_w = ec(nc.semaphore("dma_w"))
        w_ready = ec(nc.semaphore("w_ready"))
        g_mask = ec(nc.semaphore("g_mask"))
        dma_c = [ec(nc.semaphore(f"dma_c{c}")) for c in range(nchunk)]
        v_red = ec(nc.semaphore("v_red"))    # +1 per finished red[c]
        pe_sem = ec(nc.semaphore("pe_sem"))
        v_sem = ec(nc.semaphore("v_sem"))    # s_bf ready
        v_out = ec(nc.semaphore("v_out"))
        dma_out = ec(nc.semaphore("dma_out"))
        block = ec(_NoBarrierBlock(nc, f"block_{nc.next_id()}"))

        @block.sync
        def _(sync):
            for c in range(nchunk):
                sync.dma_start(
                    xc[c][:], x3[:, starts[c] : starts[c + 1], :]
                ).then_inc(dma_c[c], 16)
            sync.wait_ge(v_out, 1)
            sync.dma_start(out[:], out_sb[:]).then_inc(dma_out, 16)
            if wait_out:
                sync.wait_ge(dma_out, 16)

        @block.scalar
        def _(scalar):
            # W only feeds the final tiny matmul; the ACT HWDGE ring keeps it
            # off the sync ring, and Scalar does the bf16 cast itself.
            scalar.dma_start(w_sb[:], w[:]).then_inc(dma_w, 16)
            scalar.wait_ge(dma_w, 16)
            scalar.copy(w_bf[:], w_sb[:]).then_inc(w_ready, 1)

        @block.gpsimd
        def _(gpsimd):
            # 0/1 batch mask, one 32-partition quadrant at a time (nonzero
            # partition bases only allow 32-partition windows)
            op = None
            for q in range(4):
                for b in range(BSH):
                    op = gpsimd.memset(
                        mask_bf[32 * q : 32 * (q + 1), b : b + 1],
                        1.0 if q == b else 0.0,
                    )
            op.then_inc(g_mask, 1)

        @block.vector
        def _(vector):
            for c in range(nchunk):
                vector.wait_ge(dma_c[c], 16)
                rows = chunks[c]
                half = rows // 2 * DIN
                # fp32 -> bf16 narrowing add, then 2x-rate bf16 halvings.
                # Stop at TWO rows (PE eats the last level as a second cheap
                # bf16 matmul) so tail folds stay under the chunk arrival rate.
                op = vector.tensor_add(
                    red[c][:, :half], xc[c][:, :half],
                    xc[c][:, half : 2 * half],
                )
                s = half
                while s > _red_rows(rows) * DIN:
                    s //= 2
                    op = vector.tensor_add(
                        red[c][:, :s], red[c][:, :s], red[c][:, s : 2 * s]
                    )
                op.then_inc(v_red, 1)
            vector.wait_ge(pe_sem, 1)
            vector.tensor_copy(s_bf[:], psum_s[:]).then_inc(v_sem, 1)
            vector.wait_ge(pe_sem, 2)
            vector.tensor_copy(out_sb[:], psum_o[:]).then_inc(v_out, 1)

        @block.tensor
        def _(tensor):
            tensor.wait_ge(g_mask, 1)
            # s[d, b] += sum_p red_c[p, d] * mask[p, b], accumulated over chunks
            # (one matmul per remaining red row)
            first = True
            for c in range(nchunk):
                tensor.wait_ge(v_red, c + 1)
                for r in range(_red_rows(chunks[c])):
                    mm = tensor.matmul(
                        psum_s[:],
                        red[c][:, r * DIN : (r + 1) * DIN],
                        mask_bf[:],
                        start=first,
                        stop=(c == nchunk - 1
                              and r == _red_rows(chunks[c]) - 1),
                    )
                    first = False
            mm.then_inc(pe_sem, 1)
            tensor.wait_ge(w_ready, 1)
            tensor.wait_ge(v_sem, 1)
            # out[b, jd] = sum_d s[d, b] * W[d, jd]
            tensor.matmul(
                psum_o[:], s_bf[:], w_bf[:], start=True, stop=True
            ).then_inc(pe_sem, 1)

    return nc


def _get_nc():
    if "nc" not in _cache:
        _cache["nc"] = _build_nc()
    return _cache["nc"]


def _in_maps(x, W):
    x = np.ascontiguousarray(x, dtype=np.float32)
    W = np.ascontiguousarray(W, dtype=np.float32)
    return [{"x": x[i * BSH : (i + 1) * BSH], "W": W} for i in range(N_CORES)]


def kernel(x, W, **profile_kwargs):
    nc = _get_nc()
    res = run_bass_kernel_spmd(nc, _in_maps(x, W), list(range(N_CORES)), **profile_kwargs)
    out = np.concatenate([r["out"] for r in res.results], axis=0)
    ret = out.reshape(B, 10, 16).astype(np.float32)
    if profile_kwargs:
        ret = (ret, res)
    return ret
